# revision 12
# baseline (speedup 1.0000x reference)
"""Trainium2 Bass kernel for causal multi-head attention (16 heads, D=1024).

Sharding: tensor-parallel over heads. Each of the 8 cores owns 2 heads:
Wq/Wk/Wv split column-wise (128 cols per core), Wo split row-wise
(128 rows per core). Each core computes a full-shape partial of the
output projection; the all-reduce over partials (+ bias) happens on the
host during unsharding.

Device-side layout: everything is computed transposed.
  qT, kT = (x @ Wq_c)^T, (x @ Wk_c)^T          -> [128=2*Dh, B*S]
  scoresT[k, q] = kT-tile.T-as-lhsT @ qT        -> k on partitions
  exp on ScalarE (no max subtraction needed: |scores/8| <~ 4)
  ctxT[dh, q] + denominator row accumulated via lhsT=[v_natural | 1]
  normalize via reciprocal + gpsimd partition-broadcast
  outT_partial = Wo_c^T-chunk @ ctxT            -> [1024, B*S]

Perf notes (measured on this toolchain):
  - every lhsT change costs ~420ns serial weight load (walrus fuses
    LDWEIGHTS into fp32r matmuls; --enable-ldw-opt is disabled/broken),
    EXCEPT consecutive matmuls with identical lhsT (load elided) and
    matmuls alternating disjoint row groups (load overlaps the other
    group's stream; measured 127ns/mm for K=64 alternating h0/h1).
  - scores matmuls alternate head row groups 0:64/64:128 -> overlapped.
  - QKV runs weight-stationary over pairs of seq blocks (load elided).
  - out-proj pairs two query blocks per Wo chunk (load elided).
  - attention t-loop is software-pipelined (AV(t-1) emitted after
    scores(t)) so PE never waits on ScalarE's exp.
"""

import sys

import numpy as np

sys.path.insert(0, "/opt/trn_rl_repo")

B, S, D = 4, 2048, 1024
H, DH = 16, 64
NCORES = 8
HPC = H // NCORES            # heads per core = 2
BS = B * S                   # 8192 flattened tokens
QB = 512                     # query block (free dim of scores matmuls)
NBLK = BS // QB              # 16 projection blocks
KT = D // 128                # 8 contraction tiles for projections
NQB = S // QB                # 4 query blocks per batch
NKT = S // 128               # 16 key tiles per batch
VW = DH + 1                  # v slot width (v cols + ones col)

_BUILT = None                # cached compiled Bass module
LAST_RESULTS = None          # BassKernelResults from the last run


def _emit(tc, outT, xT, wq, wk, wv, wo, dmask, ident):
    from contextlib import ExitStack

    import concourse.tile as tile  # noqa: F401
    from concourse import mybir

    F32R = mybir.dt.float32r
    F32 = mybir.dt.float32
    Exp = mybir.ActivationFunctionType.Exp

    nc = tc.nc
    with ExitStack() as ctx:
        consts = ctx.enter_context(tc.tile_pool(name="consts", bufs=1))
        persist = ctx.enter_context(tc.tile_pool(name="persist", bufs=1))
        xpool = ctx.enter_context(tc.tile_pool(name="xpool", bufs=3))
        expp = ctx.enter_context(tc.tile_pool(name="expp", bufs=2))
        ctxp = ctx.enter_context(tc.tile_pool(name="ctxp", bufs=2))
        misc = ctx.enter_context(tc.tile_pool(name="misc", bufs=2))
        psA = ctx.enter_context(tc.tile_pool(name="psA", bufs=2, space="PSUM"))
        psB = ctx.enter_context(tc.tile_pool(name="psB", bufs=2, space="PSUM"))
        psC = ctx.enter_context(tc.tile_pool(name="psC", bufs=1, space="PSUM"))

        # ---- constants into SBUF ----
        wq_sb = consts.tile([128, KT * 128], F32R)
        wk_sb = consts.tile([128, KT * 128], F32R)
        wv_sb = consts.tile([128, KT * 128], F32R)
        wo_sb = consts.tile([128, D], F32R)
        dmask_sb = consts.tile([128, 128], F32R)
        ident_sb = consts.tile([128, 128], F32R)
        wq_v = wq.rearrange("(k p) c -> k p c", p=128)
        wk_v = wk.rearrange("(k p) c -> k p c", p=128)
        wv_v = wv.rearrange("(k p) c -> k p c", p=128)
        for kt in range(KT):
            nc.sync.dma_start(wq_sb[:, kt * 128:(kt + 1) * 128], wq_v[kt])
            nc.sync.dma_start(wk_sb[:, kt * 128:(kt + 1) * 128], wk_v[kt])
            nc.sync.dma_start(wv_sb[:, kt * 128:(kt + 1) * 128], wv_v[kt])
        nc.sync.dma_start(wo_sb[:], wo)
        nc.sync.dma_start(dmask_sb[:], dmask)
        nc.sync.dma_start(ident_sb[:], ident)

        # ---- persistent activations ----
        qT_sb = persist.tile([128, BS], F32R)   # [2*DH, B*S]
        kT_sb = persist.tile([128, BS], F32R)
        # v natural [seq, dh] tiles + ones col; slot (g*2+h) <- seq tile g,
        # head h; each slot is VW=65 cols
        v_sb = persist.tile([128, (BS // 128) * HPC * VW], F32R)
        v_view = v_sb.rearrange("p (s c) -> p s c", c=VW)
        nc.vector.memset(v_view[:, :, DH:DH + 1].bitcast(F32), 1.0)

        xT_v = xT.rearrange("(k p) n -> k p n", p=128)

        # ========== phase 1: projections, weight-stationary over 2 blocks ==
        for gp in range(NBLK // 2):
            xb = []
            for u in range(2):
                blk = gp * 2 + u
                xt = xpool.tile([128, KT * QB], F32R, name=f"xb{u}", tag="xb")
                for kt in range(KT):
                    nc.sync.dma_start(
                        xt[:, kt * QB:(kt + 1) * QB],
                        xT_v[kt, :, blk * QB:(blk + 1) * QB],
                    )
                xb.append(xt)

            for w_sb, dst in ((wq_sb, qT_sb), (wk_sb, kT_sb), (wv_sb, None)):
                ps = psA.tile([128, 2 * QB], F32, tag="s", name="ps_p")
                for kt in range(KT):
                    for u in range(2):  # same lhsT back-to-back: load elided
                        nc.tensor.matmul(
                            ps[:, u * QB:(u + 1) * QB],
                            w_sb[:, kt * 128:(kt + 1) * 128],
                            xb[u][:, kt * QB:(kt + 1) * QB],
                            start=(kt == 0),
                            stop=(kt == KT - 1),
                        )
                for u in range(2):
                    blk = gp * 2 + u
                    cols = slice(blk * QB, (blk + 1) * QB)
                    if dst is not None:
                        nc.vector.tensor_copy(dst[:, cols], ps[:, u * QB:(u + 1) * QB])
                    else:
                        # V: stage, then PE-transpose 128x128 (both heads at
                        # once) into natural layout
                        vtmp = misc.tile([128, QB], F32R, tag="vtmp")
                        nc.vector.tensor_copy(vtmp[:], ps[:, u * QB:(u + 1) * QB])
                        for cc in range(QB // 128):
                            pst = psB.tile([128, 128], F32R, tag="b", name="pst")
                            nc.tensor.transpose(
                                pst[:], vtmp[:, cc * 128:(cc + 1) * 128], ident_sb[:]
                            )
                            g = blk * (QB // 128) + cc
                            src = pst.rearrange("p (h c) -> p h c", c=DH)
                            nc.vector.tensor_copy(
                                v_view[:, g * HPC:(g + 1) * HPC, 0:DH], src
                            )

        # ========== phase 2: attention + out-proj ==========
        for b in range(B):
            cx_tiles = []
            for jq in range(NQB):
                nkt = 4 * (jq + 1)          # causal: valid key tiles
                qc = b * S + jq * QB        # query col offset in qT_sb
                ps_ctx = [
                    psC.tile([VW, QB], F32, tag=f"ctx{h}", name=f"ps_ctx{h}")
                    for h in range(HPC)
                ]
                pend = None
                for t in range(nkt):
                    kc = (b * NKT + t) * 128
                    ps_s = psA.tile([128, 2 * QB], F32, tag="s", name="ps_s")
                    for h in range(HPC):  # alternating row groups: overlapped
                        hp = slice(h * DH, (h + 1) * DH)
                        nc.tensor.matmul(
                            ps_s[:, h * QB:(h + 1) * QB],
                            kT_sb[hp, kc:kc + 128],
                            qT_sb[hp, qc:qc + QB],
                            start=True,
                            stop=True,
                        )
                    ex = expp.tile([128, 2 * QB], F32R)
                    tt = t - 4 * jq
                    c0 = 0
                    if tt < 2:  # full tile (incl. tt 0/1: cheaper unsplit)
                        nc.scalar.activation(ex[:], ps_s[:], Exp)
                    else:
                        c0 = 128 * tt
                        for h in range(HPC):
                            o = h * QB
                            nc.scalar.activation(
                                ex[:, o + c0:o + QB], ps_s[:, o + c0:o + QB], Exp
                            )
                    if tt >= 0:
                        cm = 128 * tt
                        for h in range(HPC):
                            o = h * QB
                            nc.vector.tensor_mul(
                                ex[:, o + cm:o + cm + 128],
                                ex[:, o + cm:o + cm + 128],
                                dmask_sb[:],
                            )
                    if pend is not None:
                        _av(nc, ps_ctx, v_sb, *pend, b, nkt)
                    pend = (ex, t, max(0, 128 * tt) if tt >= 0 else 0)
                _av(nc, ps_ctx, v_sb, *pend, b, nkt)

                # normalize: cx = ctx * (1/den); den is psum row DH
                cx2h = ctxp.tile([128, QB], F32R)
                for h in range(HPC):
                    den = misc.tile([128, QB], F32R, tag="den", bufs=1)
                    nc.vector.reciprocal(den[DH:DH + 1, :], ps_ctx[h][DH:DH + 1, :])
                    den0 = misc.tile([1, QB], F32R, tag="den0", bufs=1)
                    nc.sync.dma_start(den0[:], den[DH:DH + 1, :])
                    sc = misc.tile([64, QB], F32R, tag="sc", bufs=1)
                    nc.gpsimd.partition_broadcast(sc[:], den0[:])
                    if h == 0:
                        nc.vector.tensor_mul(
                            cx2h[0:DH, :], ps_ctx[h][0:DH, :], sc[:]
                        )
                    else:
                        cxs = misc.tile([64, QB], F32R, tag="cxs")
                        nc.vector.tensor_mul(cxs[:], ps_ctx[h][0:DH, :], sc[:])
                        nc.sync.dma_start(cx2h[DH:2 * DH, :], cxs[:])
                cx_tiles.append((cx2h, qc))

                # out-proj for a pair of query blocks: Wo chunk load elided
                if jq % 2 == 1:
                    for ch in range(D // 128):
                        for cx, qcc in cx_tiles:
                            ps_o = psB.tile([128, QB], F32, tag="b", name="ps_o")
                            nc.tensor.matmul(
                                ps_o[:], wo_sb[:, ch * 128:(ch + 1) * 128], cx[:],
                                start=True, stop=True,
                            )
                            ob = misc.tile([128, QB], F32R, tag="out")
                            nc.vector.tensor_copy(ob[:], ps_o[:])
                            nc.sync.dma_start(
                                outT[ch * 128:(ch + 1) * 128, qcc:qcc + QB], ob[:]
                            )
                    cx_tiles = []


def _av(nc, ps_ctx, v_sb, ex, t, c0, b, nkt):
    st, sp = (t == 0), (t == nkt - 1)
    g = b * NKT + t
    for h in range(HPC):
        o = h * QB
        s0 = (g * HPC + h) * VW
        nc.tensor.matmul(
            ps_ctx[h][:, c0:QB],
            v_sb[:, s0:s0 + VW],
            ex[:, o + c0:o + QB],
            start=st,
            stop=sp,
            skip_group_check=True,
        )


def _build(loop_n=None):
    global _BUILT
    if loop_n is None and _BUILT is not None:
        return _BUILT
    import concourse.tile as tile
    from concourse import bacc, mybir

    F32R = mybir.dt.float32r

    nc = bacc.Bacc(
        "TRN2",
        target_bir_lowering=False,
        debug=False,
        enable_asserts=False,
        num_devices=NCORES,
    )
    xT = nc.dram_tensor("xT", [D, BS], F32R, kind="ExternalInput").ap()
    wq = nc.dram_tensor("wq", [D, 128], F32R, kind="ExternalInput").ap()
    wk = nc.dram_tensor("wk", [D, 128], F32R, kind="ExternalInput").ap()
    wv = nc.dram_tensor("wv", [D, 128], F32R, kind="ExternalInput").ap()
    wo = nc.dram_tensor("wo", [128, D], F32R, kind="ExternalInput").ap()
    dmask = nc.dram_tensor("dmask", [128, 128], F32R, kind="ExternalInput").ap()
    ident = nc.dram_tensor("ident", [128, 128], F32R, kind="ExternalInput").ap()
    outT = nc.dram_tensor("outT", [D, BS], F32R, kind="ExternalOutput").ap()

    with tile.TileContext(nc) as tc:
        with nc.allow_low_precision(reason="float32r carries fp32 bits"):
            if loop_n is None:
                _emit(tc, outT, xT, wq, wk, wv, wo, dmask, ident)
            else:
                with tc.For_i(0, loop_n, 1):
                    _emit(tc, outT, xT, wq, wk, wv, wo, dmask, ident)
    nc.compile()
    if loop_n is None:
        _BUILT = nc
    return nc


def _host_inputs(x, Wq, Wk, Wv, Wo):
    """Shard + lay out the full inputs for the 8 cores."""
    x2 = np.ascontiguousarray(x.reshape(BS, D).T, dtype=np.float32)
    dmask = (np.arange(128)[None, :] >= np.arange(128)[:, None]).astype(np.float32)
    ident = np.eye(128, dtype=np.float32)
    in_maps = []
    for c in range(NCORES):
        cs = slice(c * HPC * DH, (c + 1) * HPC * DH)
        in_maps.append({
            "xT": x2,
            # fold the 1/sqrt(DH) score scale into Wq
            "wq": np.ascontiguousarray(Wq[:, cs], dtype=np.float32) / np.sqrt(DH),
            "wk": np.ascontiguousarray(Wk[:, cs], dtype=np.float32),
            "wv": np.ascontiguousarray(Wv[:, cs], dtype=np.float32),
            "wo": np.ascontiguousarray(Wo[cs, :], dtype=np.float32),
            "dmask": dmask,
            "ident": ident,
        })
    return in_maps


def kernel(x, Wq, Wk, Wv, Wo, bo):
    global LAST_RESULTS
    from concourse.bass_utils import run_bass_kernel_spmd

    nc = _build()
    in_maps = _host_inputs(
        np.asarray(x), np.asarray(Wq), np.asarray(Wk), np.asarray(Wv), np.asarray(Wo)
    )
    res = run_bass_kernel_spmd(nc, in_maps, core_ids=list(range(NCORES)))
    LAST_RESULTS = res
    acc = np.zeros((D, BS), dtype=np.float32)
    for r in res.results:
        acc += r["outT"]
    out = acc.T + np.asarray(bo, dtype=np.float32)[None, :]
    return out.reshape(B, S, D).astype(np.float32)


# revision 24
# speedup vs baseline: 1.0841x; 1.0841x over previous
"""Trainium2 Bass kernel for causal multi-head attention (16 heads, D=1024).

Sharding: tensor-parallel over heads. Each of the 8 cores owns 2 heads:
Wq/Wk/Wv split column-wise (128 cols per core), Wo split row-wise
(128 rows per core). Each core computes a full-shape partial of the
output projection; the all-reduce over partials (+ bias) happens on the
host during unsharding.

Device-side layout: everything is computed transposed.
  qT, kT = (x @ Wq_c)^T, (x @ Wk_c)^T          -> [128=2*Dh, B*S]
  scoresT[k, q] = kT-tile.T-as-lhsT @ qT        -> k on partitions
  exp on ScalarE (no max subtraction needed: |scores/8| <~ 4)
  ctxT[dh, q] + denominator row accumulated via lhsT=[v_natural | 1]
  normalize via reciprocal + gpsimd partition-broadcast
  outT_partial = Wo_c^T-chunk @ ctxT            -> [1024, B*S]

Perf notes (measured on this toolchain):
  - every lhsT change costs ~420ns serial weight load (walrus fuses
    LDWEIGHTS into fp32r matmuls; --enable-ldw-opt is disabled/broken),
    EXCEPT consecutive matmuls with identical lhsT (load elided) and
    matmuls alternating disjoint row groups (load overlaps the other
    group's stream; measured 127ns/mm for K=64 alternating h0/h1).
  - scores matmuls alternate head row groups 0:64/64:128 -> overlapped.
  - QKV runs weight-stationary over pairs of seq blocks (load elided).
  - out-proj pairs two query blocks per Wo chunk (load elided).
  - attention t-loop is software-pipelined (AV(t-1) emitted after
    scores(t)) so PE never waits on ScalarE's exp.
"""

import sys

import numpy as np

sys.path.insert(0, "/opt/trn_rl_repo")

B, S, D = 4, 2048, 1024
H, DH = 16, 64
NCORES = 8
HPC = H // NCORES            # heads per core = 2
BS = B * S                   # 8192 flattened tokens
QB = 512                     # query block (free dim of scores matmuls)
NBLK = BS // QB              # 16 projection blocks
KT = D // 128                # 8 contraction tiles for projections
NQB = S // QB                # 4 query blocks per batch
NKT = S // 128               # 16 key tiles per batch
VW = DH + 1                  # v slot width (v cols + ones col)

_BUILT = None                # cached compiled Bass module
LAST_RESULTS = None          # BassKernelResults from the last run


def _emit(tc, outT, xT, wq, wk, wv, wo, dmask, ident):
    from contextlib import ExitStack

    import concourse.tile as tile  # noqa: F401
    from concourse import mybir

    F32R = mybir.dt.float32r
    F32 = mybir.dt.float32
    Exp = mybir.ActivationFunctionType.Exp

    nc = tc.nc
    with ExitStack() as ctx:
        consts = ctx.enter_context(tc.tile_pool(name="consts", bufs=1))
        persist = ctx.enter_context(tc.tile_pool(name="persist", bufs=1))
        xpool = ctx.enter_context(tc.tile_pool(name="xpool", bufs=3))
        expp = ctx.enter_context(tc.tile_pool(name="expp", bufs=3))
        ctxp = ctx.enter_context(tc.tile_pool(name="ctxp", bufs=2))
        misc = ctx.enter_context(tc.tile_pool(name="misc", bufs=2))
        psA = ctx.enter_context(tc.tile_pool(name="psA", bufs=2, space="PSUM"))
        psB = ctx.enter_context(tc.tile_pool(name="psB", bufs=2, space="PSUM"))
        psC = ctx.enter_context(tc.tile_pool(name="psC", bufs=1, space="PSUM"))

        # ---- constants into SBUF ----
        wq_sb = consts.tile([128, KT * 128], F32R)
        wk_sb = consts.tile([128, KT * 128], F32R)
        wv_sb = consts.tile([128, KT * 128], F32R)
        wo_sb = consts.tile([128, D], F32R)
        dmask_sb = consts.tile([128, 128], F32R)
        ident_sb = consts.tile([128, 128], F32R)
        wq_v = wq.rearrange("(k p) c -> k p c", p=128)
        wk_v = wk.rearrange("(k p) c -> k p c", p=128)
        wv_v = wv.rearrange("(k p) c -> k p c", p=128)
        for kt in range(KT):
            nc.sync.dma_start(wq_sb[:, kt * 128:(kt + 1) * 128], wq_v[kt])
            nc.sync.dma_start(wk_sb[:, kt * 128:(kt + 1) * 128], wk_v[kt])
            nc.sync.dma_start(wv_sb[:, kt * 128:(kt + 1) * 128], wv_v[kt])
        nc.sync.dma_start(wo_sb[:], wo)
        nc.sync.dma_start(dmask_sb[:], dmask)
        nc.sync.dma_start(ident_sb[:], ident)

        # ---- persistent activations ----
        qT_sb = persist.tile([128, BS], F32R)   # [2*DH, B*S]
        kT_sb = persist.tile([128, BS], F32R)
        # v natural [seq, dh] tiles + ones col; slot (g*2+h) <- seq tile g,
        # head h; each slot is VW=65 cols
        v_sb = persist.tile([128, (BS // 128) * HPC * VW], F32R)
        v_view = v_sb.rearrange("p (s c) -> p s c", c=VW)
        nc.vector.memset(v_view[:, :, DH:DH + 1].bitcast(F32), 1.0)

        xT_v = xT.rearrange("(k p) n -> k p n", p=128)

        # ========== phase 1: projections, weight-stationary over 2 blocks ==
        for gp in range(NBLK // 2):
            xb = []
            for u in range(2):
                blk = gp * 2 + u
                xt = xpool.tile([128, KT * QB], F32R, name=f"xb{u}", tag="xb")
                for kt in range(KT):
                    nc.sync.dma_start(
                        xt[:, kt * QB:(kt + 1) * QB],
                        xT_v[kt, :, blk * QB:(blk + 1) * QB],
                    )
                xb.append(xt)

            for w_sb, dst in ((wq_sb, qT_sb), (wk_sb, kT_sb), (wv_sb, None)):
                ps = psA.tile([128, 2 * QB], F32, tag="s", name="ps_p")
                for kt in range(KT):
                    for u in range(2):  # same lhsT back-to-back: load elided
                        nc.tensor.matmul(
                            ps[:, u * QB:(u + 1) * QB],
                            w_sb[:, kt * 128:(kt + 1) * 128],
                            xb[u][:, kt * QB:(kt + 1) * QB],
                            start=(kt == 0),
                            stop=(kt == KT - 1),
                        )
                for u in range(2):
                    blk = gp * 2 + u
                    cols = slice(blk * QB, (blk + 1) * QB)
                    if dst is not None:
                        nc.vector.tensor_copy(dst[:, cols], ps[:, u * QB:(u + 1) * QB])
                    else:
                        # V: stage, then PE-transpose 128x128 (both heads at
                        # once) into natural layout
                        vtmp = misc.tile([128, QB], F32R, tag="vtmp")
                        nc.vector.tensor_copy(vtmp[:], ps[:, u * QB:(u + 1) * QB])
                        for cc in range(QB // 128):
                            pst = psB.tile([128, 128], F32R, tag="b", name="pst")
                            nc.tensor.transpose(
                                pst[:], vtmp[:, cc * 128:(cc + 1) * 128], ident_sb[:]
                            )
                            g = blk * (QB // 128) + cc
                            src = pst.rearrange("p (h c) -> p h c", c=DH)
                            nc.vector.tensor_copy(
                                v_view[:, g * HPC:(g + 1) * HPC, 0:DH], src
                            )

        # ========== phase 2: attention + out-proj ==========
        for b in range(B):
            cx_tiles = []
            for jq in range(NQB):
                nkt = 4 * (jq + 1)          # causal: valid key tiles
                qc = b * S + jq * QB        # query col offset in qT_sb
                ps_ctx = [
                    psC.tile([VW, QB], F32, tag=f"ctx{h}", name=f"ps_ctx{h}")
                    for h in range(HPC)
                ]
                pend = []
                for t in range(nkt):
                    kc = (b * NKT + t) * 128
                    ps_s = psA.tile([128, 2 * QB], F32, tag="s", name="ps_s")
                    for h in range(HPC):  # alternating row groups: overlapped
                        hp = slice(h * DH, (h + 1) * DH)
                        nc.tensor.matmul(
                            ps_s[:, h * QB:(h + 1) * QB],
                            kT_sb[hp, kc:kc + 128],
                            qT_sb[hp, qc:qc + QB],
                            start=True,
                            stop=True,
                        )
                    ex = expp.tile([128, 2 * QB], F32R)
                    tt = t - 4 * jq
                    c0 = 128 * tt if tt >= 0 else 0
                    if tt < 2:  # full-tile exp (tt 0/1: cheaper unsplit)
                        nc.scalar.activation(ex[:], ps_s[:], Exp)
                    else:
                        for h in range(HPC):
                            o = h * QB
                            nc.scalar.activation(
                                ex[:, o + c0:o + QB], ps_s[:, o + c0:o + QB], Exp
                            )
                    if tt >= 0:
                        for h in range(HPC):
                            o = h * QB
                            nc.vector.tensor_mul(
                                ex[:, o + c0:o + c0 + 128],
                                ex[:, o + c0:o + c0 + 128],
                                dmask_sb[:],
                            )
                    # AV parts are delayed 2 tiles (software pipeline) so PE
                    # never waits on ScalarE's exp; the masked diagonal part
                    # is split off so the DVE mask is never on PE's critical
                    # path for the wide clean part.
                    if tt >= 0 and t > 0:
                        # split off the masked diagonal block; start=False on
                        # both parts since t=0 already initialized the bank
                        if c0 + 128 < QB:
                            pend.append((ex, t, c0 + 128, QB))
                        pend.append((ex, t, c0, c0 + 128))
                    else:
                        # t == 0 must be a single start=True matmul covering
                        # the full bank (start zeroes the whole bank)
                        pend.append((ex, t, 0, QB))
                    while len(pend) > 3:
                        _av(nc, ps_ctx, v_sb, *pend.pop(0), b, False)
                for p in pend:
                    _av(nc, ps_ctx, v_sb, *p, b, p[1] == nkt - 1)

                # normalize: cx = ctx * (1/den); den is psum row DH.
                # Broadcast down the partitions via a K=1 matmul whose lhsT is
                # dmask row 64 cols 64:128 (all ones), keeping the chain short.
                cx2h = ctxp.tile([128, QB], F32R)
                for h in range(HPC):
                    den = misc.tile([128, QB], F32R, tag="den", bufs=2)
                    nc.vector.reciprocal(
                        den[DH:DH + 1, :], ps_ctx[h][DH:DH + 1, :]
                    )
                    ps_sc = psB.tile([64, QB], F32, tag="b", name="ps_sc")
                    nc.tensor.matmul(
                        ps_sc[:], dmask_sb[DH:DH + 1, DH:2 * DH],
                        den[DH:DH + 1, :], start=True, stop=True,
                        skip_group_check=True,
                    )
                    sc = misc.tile([64, QB], F32R, tag="sc", bufs=2)
                    nc.scalar.copy(sc[:], ps_sc[:])
                    if h == 0:
                        nc.vector.tensor_mul(
                            cx2h[0:DH, :], ps_ctx[h][0:DH, :], sc[:]
                        )
                    else:
                        cxs = misc.tile([64, QB], F32R, tag="cxs", bufs=1)
                        nc.vector.tensor_mul(cxs[:], ps_ctx[h][0:DH, :], sc[:])
                        nc.sync.dma_start(cx2h[DH:2 * DH, :], cxs[:])
                cx_tiles.append((cx2h, qc))

                # out-proj for a pair of query blocks: Wo chunk load elided
                if jq % 2 == 1:
                    for ch in range(D // 128):
                        for cx, qcc in cx_tiles:
                            ps_o = psB.tile([128, QB], F32, tag="b", name="ps_o")
                            nc.tensor.matmul(
                                ps_o[:], wo_sb[:, ch * 128:(ch + 1) * 128], cx[:],
                                start=True, stop=True,
                            )
                            ob = misc.tile([128, QB], F32R, tag="out")
                            nc.vector.tensor_copy(ob[:], ps_o[:])
                            nc.sync.dma_start(
                                outT[ch * 128:(ch + 1) * 128, qcc:qcc + QB], ob[:]
                            )
                    cx_tiles = []


def _av(nc, ps_ctx, v_sb, ex, t, c0, c1, b, sp):
    st = (t == 0)
    g = b * NKT + t
    for h in range(HPC):
        o = h * QB
        s0 = (g * HPC + h) * VW
        nc.tensor.matmul(
            ps_ctx[h][:, c0:c1],
            v_sb[:, s0:s0 + VW],
            ex[:, o + c0:o + c1],
            start=st,
            stop=sp,
            skip_group_check=True,
        )


def _build(loop_n=None):
    global _BUILT
    if loop_n is None and _BUILT is not None:
        return _BUILT
    import concourse.tile as tile
    from concourse import bacc, mybir

    F32R = mybir.dt.float32r

    nc = bacc.Bacc(
        "TRN2",
        target_bir_lowering=False,
        debug=False,
        enable_asserts=False,
        num_devices=NCORES,
    )
    xT = nc.dram_tensor("xT", [D, BS], F32R, kind="ExternalInput").ap()
    wq = nc.dram_tensor("wq", [D, 128], F32R, kind="ExternalInput").ap()
    wk = nc.dram_tensor("wk", [D, 128], F32R, kind="ExternalInput").ap()
    wv = nc.dram_tensor("wv", [D, 128], F32R, kind="ExternalInput").ap()
    wo = nc.dram_tensor("wo", [128, D], F32R, kind="ExternalInput").ap()
    dmask = nc.dram_tensor("dmask", [128, 128], F32R, kind="ExternalInput").ap()
    ident = nc.dram_tensor("ident", [128, 128], F32R, kind="ExternalInput").ap()
    outT = nc.dram_tensor("outT", [D, BS], F32R, kind="ExternalOutput").ap()

    with tile.TileContext(nc) as tc:
        with nc.allow_low_precision(reason="float32r carries fp32 bits"):
            if loop_n is None:
                _emit(tc, outT, xT, wq, wk, wv, wo, dmask, ident)
            else:
                with tc.For_i(0, loop_n, 1):
                    _emit(tc, outT, xT, wq, wk, wv, wo, dmask, ident)
    nc.compile()
    if loop_n is None:
        _BUILT = nc
    return nc


def _host_inputs(x, Wq, Wk, Wv, Wo):
    """Shard + lay out the full inputs for the 8 cores."""
    x2 = np.ascontiguousarray(x.reshape(BS, D).T, dtype=np.float32)
    dmask = (np.arange(128)[None, :] >= np.arange(128)[:, None]).astype(np.float32)
    ident = np.eye(128, dtype=np.float32)
    in_maps = []
    for c in range(NCORES):
        cs = slice(c * HPC * DH, (c + 1) * HPC * DH)
        in_maps.append({
            "xT": x2,
            # fold the 1/sqrt(DH) score scale into Wq
            "wq": np.ascontiguousarray(Wq[:, cs], dtype=np.float32) / np.sqrt(DH),
            "wk": np.ascontiguousarray(Wk[:, cs], dtype=np.float32),
            "wv": np.ascontiguousarray(Wv[:, cs], dtype=np.float32),
            "wo": np.ascontiguousarray(Wo[cs, :], dtype=np.float32),
            "dmask": dmask,
            "ident": ident,
        })
    return in_maps


def kernel(x, Wq, Wk, Wv, Wo, bo):
    global LAST_RESULTS
    from concourse.bass_utils import run_bass_kernel_spmd

    nc = _build()
    in_maps = _host_inputs(
        np.asarray(x), np.asarray(Wq), np.asarray(Wk), np.asarray(Wv), np.asarray(Wo)
    )
    res = run_bass_kernel_spmd(nc, in_maps, core_ids=list(range(NCORES)))
    LAST_RESULTS = res
    acc = np.zeros((D, BS), dtype=np.float32)
    for r in res.results:
        acc += r["outT"]
    out = acc.T + np.asarray(bo, dtype=np.float32)[None, :]
    return out.reshape(B, S, D).astype(np.float32)
